# revision 48
# baseline (speedup 1.0000x reference)
"""Sliding-window causal GQA attention with ALiBi for Trainium2, SPMD on 8
NeuronCores.

Problem (hardcoded): B=1, S=2048, D=2048, 16 query heads / 4 KV groups,
head_dim 128, window 512.

Sharding: tensor parallel over heads — core c owns KV group c//2 and query
head pair c%2 within that group (2 query heads per core, full sequence).
Wq/Wk/Wv are column-sharded by head, Wo row-sharded; each core produces a
full-shape partial of the output projection and the host sums the 8 partials
(replaces the all-reduce).

Device-side layout: the host passes x TRANSPOSED (xt = x.T, [D, S]). All
projections then emit transposed activations (qT/kT/vT = [head_dim, S]),
scores are computed as [keys, q] blocks — exactly the operand order the PE
array wants for the probs @ V matmul (keys on the contraction partition) —
and yT = [head_dim, q] is exactly the lhsT the output projection wants. The
only on-device transposes are 16 PE-transposes of V tiles.

Schedule (iterated against NTFF profiles; 206us -> 174us on HW):
 - few large DMA descriptors (each costs ~610ns of serial HWDGE issue): 16
   for xt (one per 512KB contraction chunk), one per weight tensor, 16 for
   the output (one per 128-row stripe) — the first revision used 194
   descriptors and spent its first 22us issuing instead of computing.
 - ~150 dependency-free warm-up matmuls bridge the ~12us engine-start +
   instruction-load + first-transfer window so the HAM clock gate has the PE
   at 2.4GHz (not 1.2) when the real work arrives.
 - projections run as two interleaved passes — (K, Q0) then (V, Q1), 8 PSUM
   banks each — so the PE chews each xt chunk as it lands from HBM.
 - attention software-pipelines score/bias matmuls one key-tile ahead of the
   PV/rowsum matmuls so the exp on the scalar engine is off the PE critical
   path.
 - softmax normalization: 1/rowsum computed as exp(-ln r) on the scalar
   engine (both funcs share one ACT table; the 3.3us serial DVE reciprocal
   FIFO-blocked every op queued behind it), broadcast across partitions with
   a rank-1 fp16 PE matmul (fp32 would double-pump LOW/HIGH), applied by one
   DVE multiply. The chain is sliced into three stages pumped at fixed
   points inside later PE instruction groups, so neither the in-order PE
   queue nor the exp-saturated ACT queue ever waits on it.
 - engine balancing: PSUM evacuations split between ACT and DVE so neither
   exceeds the PE's per-window budget; the final q-chunk stores in
   half-stripes on both HWDGE queues to shorten the drain tail.
"""

import math

import numpy as np
import ml_dtypes

import concourse.bass as bass
import concourse.mybir as mybir
import concourse.tile as tile
from concourse.masks import make_identity

BF16 = ml_dtypes.bfloat16

B, S, D = 1, 2048, 2048
NH, NKV, HD = 16, 4, 128
REP = NH // NKV          # query heads per KV group
WINDOW = 512
NCORES = 8
HPC = 2                  # query heads per core
QC = 512                 # q-chunk width (one PSUM bank of fp32)
NQC = S // QC            # 4
NKT = S // 128           # 16 key tiles
NDC = D // 128           # 16 contraction chunks
TW = WINDOW + 128        # 640: bias template width
NEG = -1.0e30

FP32 = mybir.dt.float32
BF = mybir.dt.bfloat16
F16 = mybir.dt.float16

# NOTE: reciprocal_approx_fast is a custom DVE op this container's walrus
# cannot encode ("ISA wrong length" in visitInstISA) — use stock reciprocal.


def _alibi_slopes(n_heads: int) -> np.ndarray:
    def pow2_slopes(n):
        start = 2.0 ** (-(2.0 ** (-(math.log2(n) - 3))))
        return [start * start**i for i in range(n)]

    if math.log2(n_heads).is_integer():
        slopes = pow2_slopes(n_heads)
    else:
        closest = 2 ** math.floor(math.log2(n_heads))
        slopes = pow2_slopes(closest)
        slopes += pow2_slopes(2 * closest)[0::2][: n_heads - closest]
    return np.asarray(slopes, dtype=np.float32)


def _bias_templates() -> np.ndarray:
    """[NH, 128, TW] fp32. Template col c of key-tile row kc corresponds to
    query position q = k0 + c (k0 = key tile start). Valid iff kc <= c <=
    kc + WINDOW - 1; value -slope * (c - kc), else -1e30."""
    slopes = _alibi_slopes(NH)
    kc = np.arange(128)[:, None]
    c = np.arange(TW)[None, :]
    dist = (c - kc).astype(np.float32)
    valid = (dist >= 0) & (dist <= WINDOW - 1)
    out = np.empty((NH, 128, TW), np.float32)
    for h in range(NH):
        out[h] = np.where(valid, -slopes[h] * dist, NEG)
    return out


def _split_waits(nc, maxw=1):
    """This container's walrus rejects instructions with more than one sync
    wait command; hoist extra waits onto preceding same-engine NoOps."""
    plan = {}
    si_type = None
    for bb in nc.main_func.blocks:
        for ins in bb.instructions:
            si = ins.sync_info
            waits = list(si.on_wait) if si and si.on_wait else []
            if len(waits) > maxw:
                si_type = type(si)
                extra = [waits[i:i + maxw] for i in range(0, len(waits) - maxw, maxw)]
                keep = waits[len(extra) * maxw:]
                plan[ins.name] = (extra, keep)
    if not plan:
        return 0
    nops = {}
    nop_names = set()
    for name, (extra, _keep) in plan.items():
        target = nc.inst_map[name]
        eng = nc.engines[target.engine]
        lst = []
        for chunk in extra:
            nop = eng.nop(nofuse=True).ins
            nop.sync_info = si_type(on_wait=chunk, on_update=[])
            lst.append(nop)
            nop_names.add(nop.name)
        nops[name] = lst
    for bb in nc.main_func.blocks:
        insts = list(bb.instructions)
        out = []
        changed = False
        for ins in insts:
            if ins.name in nop_names:
                changed = True
                continue
            if ins.name in plan:
                _extra, keep = plan[ins.name]
                si = ins.sync_info
                upd = list(si.on_update) if si and si.on_update else []
                ins.sync_info = si_type(on_wait=keep, on_update=upd)
                out.extend(nops[ins.name])
                changed = True
            out.append(ins)
        if changed:
            bb.instructions = out
    return len(plan)


def _kt_range(qc):
    """Key tiles feeding q-chunk qc: keys [qc*QC - WINDOW + 1, qc*QC + QC - 1]."""
    lo = max(0, (qc * QC - WINDOW + 1) // 128)
    hi = (qc * QC + QC - 1) // 128
    return lo, hi


def _build_program():
    nc = bass.Bass()

    # weight inputs arrive pre-shuffled to partition-major layouts so every
    # input DMA is one fully contiguous descriptor
    xt = nc.dram_tensor("xt", [D, S], BF, kind="ExternalInput")
    wq = nc.dram_tensor("wq", [128, NDC * HPC * HD], BF, kind="ExternalInput")
    wk = nc.dram_tensor("wk", [128, NDC * HD], BF, kind="ExternalInput")
    wv = nc.dram_tensor("wv", [128, NDC * HD], BF, kind="ExternalInput")
    wo = nc.dram_tensor("wo", [128, HPC * D], BF, kind="ExternalInput")
    biast = nc.dram_tensor("biast", [128, HPC * TW], BF, kind="ExternalInput")
    out = nc.dram_tensor("out", [S, D], F16, kind="ExternalOutput")

    with tile.TileContext(nc) as tc:
        with tc.tile_pool(name="persist", bufs=1) as persist:
            xt_sb = [persist.tile([128, S], BF, name=f"xt{d}") for d in range(NDC)]
            wq_sb = persist.tile([128, NDC * HPC * HD], BF)
            wk_sb = persist.tile([128, NDC * HD], BF)
            wv_sb = persist.tile([128, NDC * HD], BF)
            wo_sb = persist.tile([128, HPC, D], BF)
            bias_sb = persist.tile([128, HPC, TW], BF)
            qt_sb = [persist.tile([128, S], BF, name=f"qt{h}") for h in range(HPC)]
            kt_sb = persist.tile([128, S], BF)
            vt_sb = persist.tile([128, S], BF)
            v_sb = [persist.tile([128, HD], BF, name=f"v{i}") for i in range(NKT)]
            # normalized y^T per (h, qc)
            yt_sb = [
                [persist.tile([128, QC], BF, name=f"yt{h}_{q}") for q in range(NQC)]
                for h in range(HPC)
            ]
            ident = persist.tile([128, 128], BF)
            ones_k = persist.tile([128, 1], BF)
            # fp16 (not fp32) so the rank-1 broadcast matmul is a single PE
            # pass — fp32 matmuls run a LOW+HIGH double pass at ~6x the cost
            ones_row = persist.tile([1, 128], F16)
            junk = persist.tile([128, 128], BF)  # never written: warm-up operand

            # input DMAs: wk first (gates the first matmul), then xt chunks
            # on the sync queue while the remaining weights trickle in on the
            # scalar queue.
            nc.scalar.dma_start(out=wk_sb, in_=wk[:, :])
            nc.scalar.dma_start(out=wq_sb, in_=wq[:, :])
            for dch in range(NDC):
                nc.sync.dma_start(
                    out=xt_sb[dch], in_=xt[dch * 128:(dch + 1) * 128, :]
                )
            nc.scalar.dma_start(out=wv_sb, in_=wv[:, :])
            nc.scalar.dma_start(
                out=wo_sb.rearrange("p h n -> p (h n)"), in_=wo[:, :]
            )
            nc.scalar.dma_start(
                out=bias_sb.rearrange("p h c -> p (h c)"), in_=biast[:, :]
            )
            nc.vector.memset(junk, 1.0)
            make_identity(nc, ident)
            nc.vector.memset(ones_k, 1.0)
            nc.vector.memset(ones_row, 1.0)

            # warm-up matmuls: the HAM clock gate holds the PE at 1.2 GHz
            # until it has seen ~3.4us of sustained activity, and re-gates
            # after ~3.4us idle. The first real matmul can't start until
            # wk+xt0 land (~12us: engine preamble + instruction-stream load +
            # first transfers), so bridge the whole wait with dependency-free
            # matmuls on an uninitialized tile (no producer to wait for).
            with tc.tile_pool(name="warm", bufs=1, space="PSUM") as warm_pool:
                warm_ps = warm_pool.tile([128, 128], FP32, tag="w")
                for _ in range(150):
                    nc.tensor.matmul(warm_ps, junk, junk, start=True, stop=True)

            # ---- phase 1: projections ----
            # pass A interleaves kT + q0T per contraction chunk (8 PSUM
            # banks) so the PE consumes each xt chunk as it arrives from
            # HBM. v/q1 then run as sequential half-passes (xt is resident)
            # so v's accumulation stops early: its evacuation and the PE
            # transposes overlap the q1 matmuls instead of serializing at
            # the attention boundary.
            with tc.tile_pool(name="proj_ps", bufs=8, space="PSUM") as proj_ps:

                def proj_pass(jobs, lag=0):
                    # jobs: list of (w_tile, w_col0, w_stride); job 1 runs
                    # `lag` chunks behind job 0 so its weight DMA never
                    # head-of-line blocks job 0's chunk-paced matmuls
                    pss = [
                        [
                            proj_ps.tile([128, QC], FP32, tag="pp", name=f"pp{j}_{sc}")
                            for sc in range(NQC)
                        ]
                        for j in range(len(jobs))
                    ]
                    for step in range(NDC + lag):
                        for j, (w_t, col0, stride) in enumerate(jobs):
                            dch = step - (lag if j == 1 else 0)
                            if not 0 <= dch < NDC:
                                continue
                            base = dch * stride + col0
                            for sc in range(NQC):
                                nc.tensor.matmul(
                                    pss[j][sc],
                                    w_t[:, base:base + 128],
                                    xt_sb[dch][:, sc * QC:(sc + 1) * QC],
                                    start=(dch == 0),
                                    stop=(dch == NDC - 1),
                                )
                    return pss

                pss = proj_pass(
                    [(wk_sb, 0, HD), (wq_sb, 0, HPC * HD)]
                )
                for sc in range(NQC):
                    nc.scalar.copy(
                        out=kt_sb[:, sc * QC:(sc + 1) * QC], in_=pss[0][sc]
                    )
                for sc in range(NQC):
                    nc.scalar.copy(
                        out=qt_sb[0][:, sc * QC:(sc + 1) * QC], in_=pss[1][sc]
                    )

            with tc.tile_pool(name="projv_ps", bufs=4, space="PSUM") as projv_ps:
                vps = [
                    projv_ps.tile([128, QC], FP32, tag="pv", name=f"pv{sc}")
                    for sc in range(NQC)
                ]
                for dch in range(NDC):
                    for sc in range(NQC):
                        nc.tensor.matmul(
                            vps[sc],
                            wv_sb[:, dch * HD:(dch + 1) * HD],
                            xt_sb[dch][:, sc * QC:(sc + 1) * QC],
                            start=(dch == 0),
                            stop=(dch == NDC - 1),
                        )
                # v evacuation on DVE (the PE transposes wait on it; ACT
                # must stay free for the first attention exps)
                for sc in range(NQC):
                    nc.vector.tensor_copy(
                        vt_sb[:, sc * QC:(sc + 1) * QC], vps[sc]
                    )

            # transposes right after the v half-pass: the PE chews them while
            # the vt copies stream, then runs the dense q1 half-pass; all
            # boundary copies stay on the DVE so the scalar engine enters the
            # attention phase with an empty queue.
            with tc.tile_pool(name="tp_ps", bufs=2, space="PSUM") as tp_ps, \
                 tc.tile_pool(name="projq_ps", bufs=4, space="PSUM") as projq_ps:
                for sc in range(NQC):
                    for kt in range(4 * sc, 4 * sc + 4):
                        tp = tp_ps.tile([128, 128], BF, tag="tp")
                        nc.tensor.transpose(
                            tp, vt_sb[:, kt * 128:(kt + 1) * 128], ident
                        )
                        nc.vector.tensor_copy(v_sb[kt], tp)
                qps = [
                    projq_ps.tile([128, QC], FP32, tag="pq", name=f"pq{sc}")
                    for sc in range(NQC)
                ]
                for dch in range(NDC):
                    for sc in range(NQC):
                        nc.tensor.matmul(
                            qps[sc],
                            wq_sb[:, dch * HPC * HD + HD:dch * HPC * HD + HD + 128],
                            xt_sb[dch][:, sc * QC:(sc + 1) * QC],
                            start=(dch == 0),
                            stop=(dch == NDC - 1),
                        )
                for sc in range(NQC):
                    nc.vector.tensor_copy(
                        qt_sb[1][:, sc * QC:(sc + 1) * QC], qps[sc]
                    )

            # ---- phase 2: attention + output projection, per q-chunk ----
            with tc.tile_pool(name="sc_ps", bufs=2, space="PSUM") as sc_ps, \
                 tc.tile_pool(name="yps", bufs=2, space="PSUM") as y_pool, \
                 tc.tile_pool(name="nps", bufs=2, space="PSUM") as norm_ps, \
                 tc.tile_pool(name="op_ps", bufs=2, space="PSUM") as op_ps, \
                 tc.tile_pool(name="et_sb", bufs=4) as et_pool, \
                 tc.tile_pool(name="nsb", bufs=2) as norm_sb, \
                 tc.tile_pool(name="ostg", bufs=3) as ostg_pool:

                # The normalize chain is sliced into three stages, pumped one
                # at a time at fixed points inside the NEXT PE group, so that
                # (a) the in-order PE queue never head-of-line blocks on the
                # chain and (b) the ACT-engine FIFO never delays the next
                # group's exps (the scalar engine is the second-busiest
                # resource in the attention phase).
                stages = []

                def pump():
                    if stages:
                        stages.pop(0)()

                def attention(h, qc):
                    q0 = qc * QC
                    klo, khi = _kt_range(qc)
                    y_ps = y_pool.tile([128, QC], FP32, tag="y")
                    r_ps = norm_ps.tile([1, QC], FP32, tag="r")
                    # shifted-window PSUM accumulation: the first matmul
                    # (start=True) must cover all 512 columns since
                    # has_written is per-element; key tile 4*qc always does.
                    kts = [4 * qc] + [t for t in range(klo, khi + 1) if t != 4 * qc]
                    spans = []
                    for kt in kts:
                        k0 = kt * 128
                        q_lo = max(q0, k0)
                        q_hi = min(q0 + QC - 1, k0 + TW - 1)
                        spans.append((kt, k0, q_lo, q_hi - q_lo + 1))
                    ets = []
                    # software pipeline: scores/bias/exp for tile i, then
                    # PV/rowsum for tile i-1 (exp runs under the next
                    # score+bias pair)
                    for i, (kt, k0, q_lo, w) in enumerate(spans):
                        s_ps = sc_ps.tile([128, QC], FP32, tag="sc")
                        nc.tensor.matmul(
                            s_ps[:, :w],
                            kt_sb[:, kt * 128:(kt + 1) * 128],
                            qt_sb[h][:, q_lo:q_lo + w],
                            start=True,
                            stop=False,
                        )
                        nc.tensor.matmul(
                            s_ps[:, :w],
                            ident,
                            bias_sb[:, h, q_lo - k0:q_lo - k0 + w],
                            start=False,
                            stop=True,
                        )
                        et = et_pool.tile([128, QC], BF, tag="et")
                        nc.scalar.activation(
                            out=et[:, :w],
                            in_=s_ps[:, :w],
                            func=mybir.ActivationFunctionType.Exp,
                        )
                        if i in (2, 4, 6):
                            pump()
                        ets.append(et)
                        if i > 0:
                            pv(h, qc, y_ps, r_ps, spans[i - 1], ets[i - 1],
                               first=(i == 1), last=False)
                    pv(h, qc, y_ps, r_ps, spans[-1], ets[-1],
                       first=(len(spans) == 1), last=True)

                    # normalize: evacuate y to SBUF immediately (frees the
                    # PSUM bank for the next chunk's PV accumulation), recip
                    # on DVE; the rank-1 broadcast matmul + evacuate +
                    # multiply are deferred deep into the next PE group so
                    # the in-order PE queue never waits on the reciprocal.
                    # normalize chain, pumped in three stages inside the next
                    # PE group. 1/r is exp(-ln r) on the scalar engine: both
                    # funcs live in the natural_log_exp_and_others ACT table
                    # (one table load total) and each op is ~0.66us vs the
                    # 3.3us serial DVE reciprocal that FIFO-blocked every op
                    # queued behind it.
                    st = {}

                    def stage1(h=h, qc=qc, y_ps=y_ps, r_ps=r_ps, st=st):
                        yun = norm_sb.tile([128, QC], FP32, tag="yun")
                        nc.vector.tensor_copy(yun, y_ps)
                        lnr = norm_sb.tile([1, QC], FP32, tag="lnr")
                        nc.scalar.activation(
                            out=lnr, in_=r_ps,
                            func=mybir.ActivationFunctionType.Ln,
                        )
                        st["yun"], st["lnr"] = yun, lnr

                    def stage2(st=st):
                        recip = norm_sb.tile([1, QC], F16, tag="rc")
                        nc.scalar.activation(
                            out=recip, in_=st["lnr"],
                            func=mybir.ActivationFunctionType.Exp, scale=-1.0,
                        )
                        st["rc"] = recip

                    def stage3(h=h, qc=qc, st=st):
                        rb_ps = op_ps.tile([128, QC], FP32, tag="op")
                        nc.tensor.matmul(rb_ps, ones_row, st["rc"])
                        nc.vector.tensor_tensor(
                            yt_sb[h][qc], st["yun"], rb_ps, mybir.AluOpType.mult
                        )

                    stages.extend([stage1, stage2, stage3])

                def pv(h, qc, y_ps, r_ps, span, et, first, last):
                    kt, k0, q_lo, w = span
                    q0 = qc * QC
                    o = q_lo - q0
                    nc.tensor.matmul(
                        y_ps[:, o:o + w],
                        v_sb[kt],
                        et[:, :w],
                        start=first,
                        stop=last,
                        skip_group_check=True,
                    )
                    nc.tensor.matmul(
                        r_ps[:, o:o + w],
                        ones_k,
                        et[:, :w],
                        start=first,
                        stop=last,
                        skip_group_check=True,
                    )

                def outproj(qc):
                    last = qc == NQC - 1
                    for sti in range(4):
                        st = qc * 4 + sti
                        ostg = ostg_pool.tile([128, D], F16, tag="ostg")
                        for ncol in range(D // QC):
                            ps = op_ps.tile([128, QC], FP32, tag="op")
                            for h in range(HPC):
                                nc.tensor.matmul(
                                    ps,
                                    yt_sb[h][qc][:, sti * 128:(sti + 1) * 128],
                                    wo_sb[:, h, ncol * QC:(ncol + 1) * QC],
                                    start=(h == 0),
                                    stop=(h == HPC - 1),
                                )
                            dst = ostg[:, ncol * QC:(ncol + 1) * QC]
                            # DVE takes most evacuations (the ACT is nearly
                            # saturated by the attention exps + recip chain)
                            # except in the final chunk where no exps remain
                            on_act = (ncol % 2 == 0) if last else (
                                sti % 2 == 0 and ncol == 0
                            )
                            if on_act:
                                nc.scalar.copy(out=dst, in_=ps)
                            else:
                                nc.vector.tensor_copy(dst, ps)
                            if ncol == 1:
                                pump()
                            if last and ncol % 2 == 1:
                                # final q-chunk: half-stripe stores on both
                                # HWDGE queues so the tail drains ASAP
                                eng = nc.sync if ncol == 1 else nc.scalar
                                eng.dma_start(
                                    out=out[st * 128:(st + 1) * 128,
                                            (ncol - 1) * QC:(ncol + 1) * QC],
                                    in_=ostg[:, (ncol - 1) * QC:(ncol + 1) * QC],
                                )
                        if not last:
                            nc.sync.dma_start(
                                out=out[st * 128:(st + 1) * 128, :], in_=ostg
                            )

                # out-proj lags attention by one q-chunk so the normalize
                # chain's latency never backs up the PE stream
                for qc in range(NQC):
                    for h in range(HPC):
                        attention(h, qc)
                    if qc > 0:
                        outproj(qc - 1)
                outproj(NQC - 1)

    _split_waits(nc, maxw=1)
    return nc


_NC_CACHE = None


def _get_program():
    global _NC_CACHE
    if _NC_CACHE is None:
        _NC_CACHE = _build_program()
    return _NC_CACHE


def _shuffle_chunks(w, cols):
    """[D, cols] -> [128, NDC*cols] partition-major contiguous layout."""
    return np.ascontiguousarray(
        w.reshape(NDC, 128, cols).transpose(1, 0, 2).reshape(128, NDC * cols)
    )


def build_in_maps(x, Wq, Wk, Wv, Wo):
    x = np.asarray(x, np.float32)
    Wq = np.asarray(Wq, np.float32)
    Wk = np.asarray(Wk, np.float32)
    Wv = np.asarray(Wv, np.float32)
    Wo = np.asarray(Wo, np.float32)

    xt = np.ascontiguousarray(x[0].T).astype(BF16)
    wq_s = (Wq * (1.0 / math.sqrt(HD))).astype(BF16)
    wk_s = Wk.astype(BF16)
    wv_s = Wv.astype(BF16)
    wo_s = Wo.astype(BF16)
    templates = _bias_templates()

    in_maps = []
    for c in range(NCORES):
        g, hp = c // HPC, c % HPC
        heads = [g * REP + hp * HPC + r for r in range(HPC)]
        wo_rows = wo_s[heads[0] * HD:(heads[-1] + 1) * HD, :]  # [256, D]
        in_maps.append(
            {
                "xt": xt,
                "wq": _shuffle_chunks(
                    wq_s[:, heads[0] * HD:(heads[-1] + 1) * HD], HPC * HD
                ),
                "wk": _shuffle_chunks(wk_s[:, g * HD:(g + 1) * HD], HD),
                "wv": _shuffle_chunks(wv_s[:, g * HD:(g + 1) * HD], HD),
                "wo": np.ascontiguousarray(
                    wo_rows.reshape(HPC, 128, D).transpose(1, 0, 2).reshape(128, HPC * D)
                ),
                "biast": np.ascontiguousarray(
                    templates[heads].transpose(1, 0, 2).reshape(128, HPC * TW)
                ).astype(BF16),
            }
        )
    return in_maps


_last_in_maps = None


def kernel(x, Wq, Wk, Wv, Wo):
    from concourse.bass_utils import run_bass_kernel_spmd

    global _last_in_maps
    in_maps = build_in_maps(x, Wq, Wk, Wv, Wo)
    _last_in_maps = in_maps

    nc = _get_program()
    res = run_bass_kernel_spmd(nc, in_maps, list(range(NCORES)))
    acc = res.results[0]["out"].astype(np.float64)
    for c in range(1, NCORES):
        acc += res.results[c]["out"]
    return acc.astype(np.float32).reshape(B, S, D)


# revision 49
# speedup vs baseline: 1.0092x; 1.0092x over previous
"""Sliding-window causal GQA attention with ALiBi for Trainium2, SPMD on 8
NeuronCores.

Problem (hardcoded): B=1, S=2048, D=2048, 16 query heads / 4 KV groups,
head_dim 128, window 512.

Sharding: tensor parallel over heads — core c owns KV group c//2 and query
head pair c%2 within that group (2 query heads per core, full sequence).
Wq/Wk/Wv are column-sharded by head, Wo row-sharded; each core produces a
full-shape partial of the output projection and the host sums the 8 partials
(replaces the all-reduce).

Device-side layout: the host passes x TRANSPOSED (xt = x.T, [D, S]). All
projections then emit transposed activations (qT/kT/vT = [head_dim, S]),
scores are computed as [keys, q] blocks — exactly the operand order the PE
array wants for the probs @ V matmul (keys on the contraction partition) —
and yT = [head_dim, q] is exactly the lhsT the output projection wants. The
only on-device transposes are 16 PE-transposes of V tiles.

Schedule (iterated against NTFF profiles; 206us -> 174us on HW):
 - few large DMA descriptors (each costs ~610ns of serial HWDGE issue): 16
   for xt (one per 512KB contraction chunk), one per weight tensor, 16 for
   the output (one per 128-row stripe) — the first revision used 194
   descriptors and spent its first 22us issuing instead of computing.
 - ~150 dependency-free warm-up matmuls bridge the ~12us engine-start +
   instruction-load + first-transfer window so the HAM clock gate has the PE
   at 2.4GHz (not 1.2) when the real work arrives.
 - projections run as two interleaved passes — (K, Q0) then (V, Q1), 8 PSUM
   banks each — so the PE chews each xt chunk as it lands from HBM.
 - attention software-pipelines score/bias matmuls one key-tile ahead of the
   PV/rowsum matmuls so the exp on the scalar engine is off the PE critical
   path.
 - softmax normalization: 1/rowsum computed as exp(-ln r) on the scalar
   engine (both funcs share one ACT table; the 3.3us serial DVE reciprocal
   FIFO-blocked every op queued behind it), broadcast across partitions with
   a rank-1 fp16 PE matmul (fp32 would double-pump LOW/HIGH), applied by one
   DVE multiply. The chain is sliced into three stages pumped at fixed
   points inside later PE instruction groups, so neither the in-order PE
   queue nor the exp-saturated ACT queue ever waits on it.
 - engine balancing: PSUM evacuations split between ACT and DVE so neither
   exceeds the PE's per-window budget; the final q-chunk stores in
   half-stripes on both HWDGE queues to shorten the drain tail.
"""

import math

import numpy as np
import ml_dtypes

import concourse.bass as bass
import concourse.mybir as mybir
import concourse.tile as tile
from concourse.masks import make_identity

BF16 = ml_dtypes.bfloat16

B, S, D = 1, 2048, 2048
NH, NKV, HD = 16, 4, 128
REP = NH // NKV          # query heads per KV group
WINDOW = 512
NCORES = 8
HPC = 2                  # query heads per core
QC = 512                 # q-chunk width (one PSUM bank of fp32)
NQC = S // QC            # 4
NKT = S // 128           # 16 key tiles
NDC = D // 128           # 16 contraction chunks
TW = WINDOW + 128        # 640: bias template width
NEG = -1.0e30

FP32 = mybir.dt.float32
BF = mybir.dt.bfloat16
F16 = mybir.dt.float16

# NOTE: reciprocal_approx_fast is a custom DVE op this container's walrus
# cannot encode ("ISA wrong length" in visitInstISA) — use stock reciprocal.


def _alibi_slopes(n_heads: int) -> np.ndarray:
    def pow2_slopes(n):
        start = 2.0 ** (-(2.0 ** (-(math.log2(n) - 3))))
        return [start * start**i for i in range(n)]

    if math.log2(n_heads).is_integer():
        slopes = pow2_slopes(n_heads)
    else:
        closest = 2 ** math.floor(math.log2(n_heads))
        slopes = pow2_slopes(closest)
        slopes += pow2_slopes(2 * closest)[0::2][: n_heads - closest]
    return np.asarray(slopes, dtype=np.float32)


def _bias_templates() -> np.ndarray:
    """[NH, 128, TW] fp32. Template col c of key-tile row kc corresponds to
    query position q = k0 + c (k0 = key tile start). Valid iff kc <= c <=
    kc + WINDOW - 1; value -slope * (c - kc), else -1e30."""
    slopes = _alibi_slopes(NH)
    kc = np.arange(128)[:, None]
    c = np.arange(TW)[None, :]
    dist = (c - kc).astype(np.float32)
    valid = (dist >= 0) & (dist <= WINDOW - 1)
    out = np.empty((NH, 128, TW), np.float32)
    for h in range(NH):
        out[h] = np.where(valid, -slopes[h] * dist, NEG)
    return out


def _split_waits(nc, maxw=1):
    """This container's walrus rejects instructions with more than one sync
    wait command; hoist extra waits onto preceding same-engine NoOps."""
    plan = {}
    si_type = None
    for bb in nc.main_func.blocks:
        for ins in bb.instructions:
            si = ins.sync_info
            waits = list(si.on_wait) if si and si.on_wait else []
            if len(waits) > maxw:
                si_type = type(si)
                extra = [waits[i:i + maxw] for i in range(0, len(waits) - maxw, maxw)]
                keep = waits[len(extra) * maxw:]
                plan[ins.name] = (extra, keep)
    if not plan:
        return 0
    nops = {}
    nop_names = set()
    for name, (extra, _keep) in plan.items():
        target = nc.inst_map[name]
        eng = nc.engines[target.engine]
        lst = []
        for chunk in extra:
            nop = eng.nop(nofuse=True).ins
            nop.sync_info = si_type(on_wait=chunk, on_update=[])
            lst.append(nop)
            nop_names.add(nop.name)
        nops[name] = lst
    for bb in nc.main_func.blocks:
        insts = list(bb.instructions)
        out = []
        changed = False
        for ins in insts:
            if ins.name in nop_names:
                changed = True
                continue
            if ins.name in plan:
                _extra, keep = plan[ins.name]
                si = ins.sync_info
                upd = list(si.on_update) if si and si.on_update else []
                ins.sync_info = si_type(on_wait=keep, on_update=upd)
                out.extend(nops[ins.name])
                changed = True
            out.append(ins)
        if changed:
            bb.instructions = out
    return len(plan)


def _kt_range(qc):
    """Key tiles feeding q-chunk qc: keys [qc*QC - WINDOW + 1, qc*QC + QC - 1]."""
    lo = max(0, (qc * QC - WINDOW + 1) // 128)
    hi = (qc * QC + QC - 1) // 128
    return lo, hi


def _build_program():
    nc = bass.Bass()

    # weight inputs arrive pre-shuffled to partition-major layouts so every
    # input DMA is one fully contiguous descriptor
    xt = nc.dram_tensor("xt", [D, S], BF, kind="ExternalInput")
    wq = nc.dram_tensor("wq", [128, NDC * HPC * HD], BF, kind="ExternalInput")
    wk = nc.dram_tensor("wk", [128, NDC * HD], BF, kind="ExternalInput")
    wv = nc.dram_tensor("wv", [128, NDC * HD], BF, kind="ExternalInput")
    wo = nc.dram_tensor("wo", [128, HPC * D], BF, kind="ExternalInput")
    biast = nc.dram_tensor("biast", [128, HPC * TW], BF, kind="ExternalInput")
    out = nc.dram_tensor("out", [S, D], F16, kind="ExternalOutput")

    with tile.TileContext(nc) as tc:
        with tc.tile_pool(name="persist", bufs=1) as persist:
            xt_sb = [persist.tile([128, S], BF, name=f"xt{d}") for d in range(NDC)]
            wq_sb = persist.tile([128, NDC * HPC * HD], BF)
            wk_sb = persist.tile([128, NDC * HD], BF)
            wv_sb = persist.tile([128, NDC * HD], BF)
            wo_sb = persist.tile([128, HPC, D], BF)
            bias_sb = persist.tile([128, HPC, TW], BF)
            qt_sb = [persist.tile([128, S], BF, name=f"qt{h}") for h in range(HPC)]
            kt_sb = persist.tile([128, S], BF)
            vt_sb = persist.tile([128, S], BF)
            v_sb = [persist.tile([128, HD], BF, name=f"v{i}") for i in range(NKT)]
            # normalized y^T per (h, qc)
            yt_sb = [
                [persist.tile([128, QC], BF, name=f"yt{h}_{q}") for q in range(NQC)]
                for h in range(HPC)
            ]
            ident = persist.tile([128, 128], BF)
            ones_k = persist.tile([128, 1], BF)
            # fp16 (not fp32) so the rank-1 broadcast matmul is a single PE
            # pass — fp32 matmuls run a LOW+HIGH double pass at ~6x the cost
            ones_row = persist.tile([1, 128], F16)
            junk = persist.tile([128, 128], BF)  # never written: warm-up operand

            # input DMAs: wk first (gates the first matmul), then xt chunks
            # on the sync queue while the remaining weights trickle in on the
            # scalar queue.
            nc.scalar.dma_start(out=wk_sb, in_=wk[:, :])
            nc.scalar.dma_start(out=wq_sb, in_=wq[:, :])
            for dch in range(NDC):
                nc.sync.dma_start(
                    out=xt_sb[dch], in_=xt[dch * 128:(dch + 1) * 128, :]
                )
            nc.scalar.dma_start(out=wv_sb, in_=wv[:, :])
            nc.scalar.dma_start(
                out=wo_sb.rearrange("p h n -> p (h n)"), in_=wo[:, :]
            )
            nc.scalar.dma_start(
                out=bias_sb.rearrange("p h c -> p (h c)"), in_=biast[:, :]
            )
            nc.vector.memset(junk, 1.0)
            make_identity(nc, ident)
            nc.vector.memset(ones_k, 1.0)
            nc.vector.memset(ones_row, 1.0)

            # warm-up matmuls: the HAM clock gate holds the PE at 1.2 GHz
            # until it has seen ~3.4us of sustained activity, and re-gates
            # after ~3.4us idle. The first real matmul can't start until
            # wk+xt0 land (~12us: engine preamble + instruction-stream load +
            # first transfers), so bridge the whole wait with dependency-free
            # matmuls on an uninitialized tile (no producer to wait for).
            with tc.tile_pool(name="warm", bufs=1, space="PSUM") as warm_pool:
                warm_ps = warm_pool.tile([128, 128], FP32, tag="w")
                for _ in range(150):
                    nc.tensor.matmul(warm_ps, junk, junk, start=True, stop=True)

            # ---- phase 1: projections, two interleaved passes ----
            # pass A: kT + q0T, pass B: vT + q1T. 8 PSUM banks per pass; the
            # per-dch interleave lets the PE consume each xt chunk as it
            # arrives from HBM.
            with tc.tile_pool(name="proj_ps", bufs=8, space="PSUM") as proj_ps:

                def proj_pass(jobs, lag=0):
                    # jobs: list of (w_tile, w_col0, w_stride); job 1 runs
                    # `lag` chunks behind job 0 so its weight DMA never
                    # head-of-line blocks job 0's chunk-paced matmuls
                    pss = [
                        [
                            proj_ps.tile([128, QC], FP32, tag="pp", name=f"pp{j}_{sc}")
                            for sc in range(NQC)
                        ]
                        for j in range(len(jobs))
                    ]
                    for step in range(NDC + lag):
                        for j, (w_t, col0, stride) in enumerate(jobs):
                            dch = step - (lag if j == 1 else 0)
                            if not 0 <= dch < NDC:
                                continue
                            base = dch * stride + col0
                            for sc in range(NQC):
                                nc.tensor.matmul(
                                    pss[j][sc],
                                    w_t[:, base:base + 128],
                                    xt_sb[dch][:, sc * QC:(sc + 1) * QC],
                                    start=(dch == 0),
                                    stop=(dch == NDC - 1),
                                )
                    return pss

                pss = proj_pass(
                    [(wk_sb, 0, HD), (wq_sb, 0, HPC * HD)]
                )
                for sc in range(NQC):
                    nc.scalar.copy(
                        out=kt_sb[:, sc * QC:(sc + 1) * QC], in_=pss[0][sc]
                    )
                for sc in range(NQC):
                    nc.scalar.copy(
                        out=qt_sb[0][:, sc * QC:(sc + 1) * QC], in_=pss[1][sc]
                    )

                pss = proj_pass(
                    [(wv_sb, 0, HD), (wq_sb, HD, HPC * HD)]
                )
                # v copies on DVE (the PE transposes wait on them; ACT must
                # stay free for the first attention exps), q1 on ACT
                for sc in range(NQC):
                    nc.vector.tensor_copy(
                        vt_sb[:, sc * QC:(sc + 1) * QC], pss[0][sc]
                    )
                for sc in range(NQC):
                    nc.scalar.copy(
                        out=qt_sb[1][:, sc * QC:(sc + 1) * QC], in_=pss[1][sc]
                    )

            # V tiles PE-transposed, pipelined per 512-column chunk behind the
            # vt PSUM-evacuation copies (xbar DMA transposes measured 1.2us
            # each and serialize — worse)
            with tc.tile_pool(name="tp_ps", bufs=2, space="PSUM") as tp_ps:
                for sc in range(NQC):
                    for kt in range(4 * sc, 4 * sc + 4):
                        tp = tp_ps.tile([128, 128], BF, tag="tp")
                        nc.tensor.transpose(
                            tp, vt_sb[:, kt * 128:(kt + 1) * 128], ident
                        )
                        nc.vector.tensor_copy(v_sb[kt], tp)

            # ---- phase 2: attention + output projection, per q-chunk ----
            with tc.tile_pool(name="sc_ps", bufs=2, space="PSUM") as sc_ps, \
                 tc.tile_pool(name="yps", bufs=2, space="PSUM") as y_pool, \
                 tc.tile_pool(name="nps", bufs=2, space="PSUM") as norm_ps, \
                 tc.tile_pool(name="op_ps", bufs=2, space="PSUM") as op_ps, \
                 tc.tile_pool(name="et_sb", bufs=4) as et_pool, \
                 tc.tile_pool(name="nsb", bufs=2) as norm_sb, \
                 tc.tile_pool(name="ostg", bufs=3) as ostg_pool:

                # The normalize chain is sliced into three stages, pumped one
                # at a time at fixed points inside the NEXT PE group, so that
                # (a) the in-order PE queue never head-of-line blocks on the
                # chain and (b) the ACT-engine FIFO never delays the next
                # group's exps (the scalar engine is the second-busiest
                # resource in the attention phase).
                stages = []

                def pump():
                    if stages:
                        stages.pop(0)()

                def attention(h, qc):
                    q0 = qc * QC
                    klo, khi = _kt_range(qc)
                    y_ps = y_pool.tile([128, QC], FP32, tag="y")
                    r_ps = norm_ps.tile([1, QC], FP32, tag="r")
                    # shifted-window PSUM accumulation: the first matmul
                    # (start=True) must cover all 512 columns since
                    # has_written is per-element; key tile 4*qc always does.
                    kts = [4 * qc] + [t for t in range(klo, khi + 1) if t != 4 * qc]
                    spans = []
                    for kt in kts:
                        k0 = kt * 128
                        q_lo = max(q0, k0)
                        q_hi = min(q0 + QC - 1, k0 + TW - 1)
                        spans.append((kt, k0, q_lo, q_hi - q_lo + 1))
                    ets = []
                    # software pipeline: scores/bias/exp for tile i, then
                    # PV/rowsum for tile i-1 (exp runs under the next
                    # score+bias pair)
                    for i, (kt, k0, q_lo, w) in enumerate(spans):
                        s_ps = sc_ps.tile([128, QC], FP32, tag="sc")
                        nc.tensor.matmul(
                            s_ps[:, :w],
                            kt_sb[:, kt * 128:(kt + 1) * 128],
                            qt_sb[h][:, q_lo:q_lo + w],
                            start=True,
                            stop=False,
                        )
                        nc.tensor.matmul(
                            s_ps[:, :w],
                            ident,
                            bias_sb[:, h, q_lo - k0:q_lo - k0 + w],
                            start=False,
                            stop=True,
                        )
                        et = et_pool.tile([128, QC], BF, tag="et")
                        nc.scalar.activation(
                            out=et[:, :w],
                            in_=s_ps[:, :w],
                            func=mybir.ActivationFunctionType.Exp,
                        )
                        if i in (2, 4, 6):
                            pump()
                        ets.append(et)
                        if i > 0:
                            pv(h, qc, y_ps, r_ps, spans[i - 1], ets[i - 1],
                               first=(i == 1), last=False)
                    pv(h, qc, y_ps, r_ps, spans[-1], ets[-1],
                       first=(len(spans) == 1), last=True)

                    # normalize: evacuate y to SBUF immediately (frees the
                    # PSUM bank for the next chunk's PV accumulation), recip
                    # on DVE; the rank-1 broadcast matmul + evacuate +
                    # multiply are deferred deep into the next PE group so
                    # the in-order PE queue never waits on the reciprocal.
                    # normalize chain, pumped in three stages inside the next
                    # PE group. 1/r is exp(-ln r) on the scalar engine: both
                    # funcs live in the natural_log_exp_and_others ACT table
                    # (one table load total) and each op is ~0.66us vs the
                    # 3.3us serial DVE reciprocal that FIFO-blocked every op
                    # queued behind it.
                    st = {}

                    def stage1(h=h, qc=qc, y_ps=y_ps, r_ps=r_ps, st=st):
                        yun = norm_sb.tile([128, QC], FP32, tag="yun")
                        nc.vector.tensor_copy(yun, y_ps)
                        lnr = norm_sb.tile([1, QC], FP32, tag="lnr")
                        nc.scalar.activation(
                            out=lnr, in_=r_ps,
                            func=mybir.ActivationFunctionType.Ln,
                        )
                        st["yun"], st["lnr"] = yun, lnr

                    def stage2(st=st):
                        recip = norm_sb.tile([1, QC], F16, tag="rc")
                        nc.scalar.activation(
                            out=recip, in_=st["lnr"],
                            func=mybir.ActivationFunctionType.Exp, scale=-1.0,
                        )
                        st["rc"] = recip

                    def stage3(h=h, qc=qc, st=st):
                        rb_ps = op_ps.tile([128, QC], FP32, tag="op")
                        nc.tensor.matmul(rb_ps, ones_row, st["rc"])
                        nc.vector.tensor_tensor(
                            yt_sb[h][qc], st["yun"], rb_ps, mybir.AluOpType.mult
                        )

                    stages.extend([stage1, stage2, stage3])

                def pv(h, qc, y_ps, r_ps, span, et, first, last):
                    kt, k0, q_lo, w = span
                    q0 = qc * QC
                    o = q_lo - q0
                    nc.tensor.matmul(
                        y_ps[:, o:o + w],
                        v_sb[kt],
                        et[:, :w],
                        start=first,
                        stop=last,
                        skip_group_check=True,
                    )
                    nc.tensor.matmul(
                        r_ps[:, o:o + w],
                        ones_k,
                        et[:, :w],
                        start=first,
                        stop=last,
                        skip_group_check=True,
                    )

                def outproj(qc):
                    last = qc == NQC - 1
                    for sti in range(4):
                        st = qc * 4 + sti
                        ostg = ostg_pool.tile([128, D], F16, tag="ostg")
                        for ncol in range(D // QC):
                            ps = op_ps.tile([128, QC], FP32, tag="op")
                            for h in range(HPC):
                                nc.tensor.matmul(
                                    ps,
                                    yt_sb[h][qc][:, sti * 128:(sti + 1) * 128],
                                    wo_sb[:, h, ncol * QC:(ncol + 1) * QC],
                                    start=(h == 0),
                                    stop=(h == HPC - 1),
                                )
                            dst = ostg[:, ncol * QC:(ncol + 1) * QC]
                            # DVE takes most evacuations (the ACT is nearly
                            # saturated by the attention exps + recip chain)
                            # except in the final chunk where no exps remain
                            on_act = (ncol % 2 == 0) if last else (
                                sti % 2 == 0 and ncol == 0
                            )
                            if on_act:
                                nc.scalar.copy(out=dst, in_=ps)
                            else:
                                nc.vector.tensor_copy(dst, ps)
                            if ncol == 1:
                                pump()
                            if last and ncol % 2 == 1:
                                # final q-chunk: half-stripe stores on both
                                # HWDGE queues so the tail drains ASAP
                                eng = nc.sync if ncol == 1 else nc.scalar
                                eng.dma_start(
                                    out=out[st * 128:(st + 1) * 128,
                                            (ncol - 1) * QC:(ncol + 1) * QC],
                                    in_=ostg[:, (ncol - 1) * QC:(ncol + 1) * QC],
                                )
                        if not last:
                            nc.sync.dma_start(
                                out=out[st * 128:(st + 1) * 128, :], in_=ostg
                            )

                # out-proj lags attention by one q-chunk so the normalize
                # chain's latency never backs up the PE stream
                for qc in range(NQC):
                    for h in range(HPC):
                        attention(h, qc)
                    if qc > 0:
                        outproj(qc - 1)
                outproj(NQC - 1)

    _split_waits(nc, maxw=1)
    return nc


_NC_CACHE = None


def _get_program():
    global _NC_CACHE
    if _NC_CACHE is None:
        _NC_CACHE = _build_program()
    return _NC_CACHE


def _shuffle_chunks(w, cols):
    """[D, cols] -> [128, NDC*cols] partition-major contiguous layout."""
    return np.ascontiguousarray(
        w.reshape(NDC, 128, cols).transpose(1, 0, 2).reshape(128, NDC * cols)
    )


def build_in_maps(x, Wq, Wk, Wv, Wo):
    x = np.asarray(x, np.float32)
    Wq = np.asarray(Wq, np.float32)
    Wk = np.asarray(Wk, np.float32)
    Wv = np.asarray(Wv, np.float32)
    Wo = np.asarray(Wo, np.float32)

    xt = np.ascontiguousarray(x[0].T).astype(BF16)
    wq_s = (Wq * (1.0 / math.sqrt(HD))).astype(BF16)
    wk_s = Wk.astype(BF16)
    wv_s = Wv.astype(BF16)
    wo_s = Wo.astype(BF16)
    templates = _bias_templates()

    in_maps = []
    for c in range(NCORES):
        g, hp = c // HPC, c % HPC
        heads = [g * REP + hp * HPC + r for r in range(HPC)]
        wo_rows = wo_s[heads[0] * HD:(heads[-1] + 1) * HD, :]  # [256, D]
        in_maps.append(
            {
                "xt": xt,
                "wq": _shuffle_chunks(
                    wq_s[:, heads[0] * HD:(heads[-1] + 1) * HD], HPC * HD
                ),
                "wk": _shuffle_chunks(wk_s[:, g * HD:(g + 1) * HD], HD),
                "wv": _shuffle_chunks(wv_s[:, g * HD:(g + 1) * HD], HD),
                "wo": np.ascontiguousarray(
                    wo_rows.reshape(HPC, 128, D).transpose(1, 0, 2).reshape(128, HPC * D)
                ),
                "biast": np.ascontiguousarray(
                    templates[heads].transpose(1, 0, 2).reshape(128, HPC * TW)
                ).astype(BF16),
            }
        )
    return in_maps


_last_in_maps = None


def kernel(x, Wq, Wk, Wv, Wo):
    from concourse.bass_utils import run_bass_kernel_spmd

    global _last_in_maps
    in_maps = build_in_maps(x, Wq, Wk, Wv, Wo)
    _last_in_maps = in_maps

    nc = _get_program()
    res = run_bass_kernel_spmd(nc, in_maps, list(range(NCORES)))
    acc = res.results[0]["out"].astype(np.float64)
    for c in range(1, NCORES):
        acc += res.results[c]["out"]
    return acc.astype(np.float32).reshape(B, S, D)
